# revision 15
# baseline (speedup 1.0000x reference)
import sys
import os

sys.path.insert(0, "/opt/trn_rl_repo")

import numpy as np

from concourse import bacc, mybir, tile
from concourse.bass_utils import run_bass_kernel_spmd
from concourse.masks import make_identity

B, T, D, H = 32, 512, 512, 1024
NCORES = 8
HS = H // NCORES          # 128 hidden columns owned per core
G = 4 * HS                # 512 gate columns per core: [i | f | o | g]
KV = H // 128             # 8 K-chunks for the V matmul
KU = D // 128             # 4 K-chunks for the U matmul

F32 = mybir.dt.float32
F32R = mybir.dt.float32r
SIG = mybir.ActivationFunctionType.Sigmoid
TANH = mybir.ActivationFunctionType.Tanh


def build(t_steps=T):
    nc = bacc.Bacc("TRN2", target_bir_lowering=False, debug=False, num_devices=NCORES)

    x_dr = nc.dram_tensor("x", [B, t_steps, D], F32, kind="ExternalInput")
    h0T_dr = nc.dram_tensor("h0T", [H, B], F32R, kind="ExternalInput")
    c0_dr = nc.dram_tensor("c0s", [B, HS], F32, kind="ExternalInput")
    V_dr = nc.dram_tensor("Vsel", [H, G], F32R, kind="ExternalInput")
    U_dr = nc.dram_tensor("Usel", [D, G], F32R, kind="ExternalInput")
    b_dr = nc.dram_tensor("bsel", [1, G], F32R, kind="ExternalInput")
    out_dr = nc.dram_tensor("hseq", [B, t_steps, HS], F32, kind="ExternalOutput")

    with tile.TileContext(nc) as tc:
        with (
            tc.tile_pool(name="const", bufs=1) as cpool,
            tc.tile_pool(name="xin", bufs=3) as xpool,
            tc.tile_pool(name="work", bufs=2) as wpool,
            tc.tile_pool(name="ps", bufs=2, space="PSUM") as pspool,
            tc.tile_pool(name="dram", bufs=2, space="DRAM") as dpool,
        ):
            V_sb = cpool.tile([128, KV * G], F32R)
            U_sb = cpool.tile([128, KU * G], F32R)
            b_sb = cpool.tile([1, G], F32R)
            ones_f32 = cpool.tile([1, B], F32)
            ones_sb = cpool.tile([1, B], F32R)
            ident = cpool.tile([128, 128], F32)
            c_sb = cpool.tile([B, HS], F32)

            nc.sync.dma_start(
                V_sb[:].rearrange("p (c n) -> p c n", c=KV),
                V_dr.rearrange("(c p) n -> p c n", p=128),
            )
            nc.sync.dma_start(
                U_sb[:].rearrange("p (c n) -> p c n", c=KU),
                U_dr.rearrange("(c p) n -> p c n", p=128),
            )
            nc.sync.dma_start(b_sb[:], b_dr[:])
            nc.sync.dma_start(c_sb[:], c0_dr[:])
            nc.vector.memset(ones_f32[:], 1.0)
            nc.scalar.copy(ones_sb[:], ones_f32[:])
            make_identity(nc, ident[:])

            hT_sb = wpool.tile([128, KV * B], F32R, name="hT_sb")
            nc.sync.dma_start(
                hT_sb[:].rearrange("p (c b) -> p c b", c=KV),
                h0T_dr.rearrange("(c p) b -> p c b", p=128),
            )

            for t in range(t_steps):
                x_sb = xpool.tile([B, D], F32, name="x_sb")
                nc.sync.dma_start(x_sb[:], x_dr[:, t, :])

                xT_ps = pspool.tile([128, KU * B], F32, name="xT_ps")
                for c in range(KU):
                    nc.tensor.transpose(
                        xT_ps[:, c * B:(c + 1) * B],
                        x_sb[:, c * 128:(c + 1) * 128],
                        ident[0:B, 0:B],
                    )
                xT_sb = wpool.tile([128, KU * B], F32R, name="xT_sb")
                nc.scalar.copy(xT_sb[:], xT_ps[:])

                gates_ps = pspool.tile([B, G], F32, name="gates_ps")
                nc.tensor.matmul(
                    gates_ps[:],
                    ones_sb[:],
                    b_sb[:],
                    start=True,
                    stop=False,
                )
                for c in range(KU):
                    nc.tensor.matmul(
                        gates_ps[:],
                        xT_sb[:, c * B:(c + 1) * B],
                        U_sb[:, c * G:(c + 1) * G],
                        start=False,
                        stop=False,
                    )
                for c in range(KV):
                    nc.tensor.matmul(
                        gates_ps[:],
                        hT_sb[:, c * B:(c + 1) * B],
                        V_sb[:, c * G:(c + 1) * G],
                        start=False,
                        stop=(c == KV - 1),
                    )

                gact = wpool.tile([B, G], F32, name="gact")
                nc.scalar.activation(gact[:, 0:3 * HS], gates_ps[:, 0:3 * HS], SIG)
                nc.scalar.activation(gact[:, 3 * HS:G], gates_ps[:, 3 * HS:G], TANH)

                ig_sb = wpool.tile([B, HS], F32, name="ig_sb")
                nc.vector.tensor_mul(ig_sb[:], gact[:, 0:HS], gact[:, 3 * HS:G])
                fc_sb = wpool.tile([B, HS], F32, name="fc_sb")
                nc.vector.tensor_mul(fc_sb[:], gact[:, HS:2 * HS], c_sb[:])
                nc.vector.tensor_add(c_sb[:], fc_sb[:], ig_sb[:])
                tch_sb = wpool.tile([B, HS], F32, name="tch_sb")
                nc.scalar.activation(tch_sb[:], c_sb[:], TANH)
                h_sb = wpool.tile([B, HS], F32, name="h_sb")
                nc.vector.tensor_mul(h_sb[:], gact[:, 2 * HS:3 * HS], tch_sb[:])

                nc.scalar.dma_start(out_dr[:, t, :], h_sb[:])

                if t < t_steps - 1:
                    hT_ps = pspool.tile([128, B], F32, name="hT_ps")
                    nc.tensor.transpose(hT_ps[:], h_sb[:], ident[0:B, 0:B])
                    send_sb = wpool.tile([HS, B], F32R, name="send_sb")
                    nc.scalar.copy(send_sb[:], hT_ps[:])
                    send_dr = dpool.tile([HS, B], F32R, name="send_dr")
                    nc.scalar.dma_start(send_dr[:], send_sb[:])
                    gath_dr = dpool.tile(
                        [H, B], F32R, name="gath_dr", addr_space="Shared"
                    )
                    nc.gpsimd.collective_compute(
                        "AllGather",
                        mybir.AluOpType.bypass,
                        replica_groups=[list(range(NCORES))],
                        ins=[send_dr.opt()],
                        outs=[gath_dr.opt()],
                    )
                    hT_sb = wpool.tile([128, KV * B], F32R, name="hT_sb")
                    nc.sync.dma_start(
                        hT_sb[:].rearrange("p (c b) -> p c b", c=KV),
                        gath_dr[:].rearrange("(c p) b -> p c b", p=128),
                    )

    nc.compile()
    return nc


_cache = {}
TRACE = False
LAST_EXEC_NS = None
LAST_RESULT = None


def _timed_run(nc, in_maps, n_cores, iters=10):
    """Replicates bass2jax.run_bass_via_pjrt but keeps the jitted
    executable and device-resident inputs, timing warm executions."""
    import time as _time

    import jax
    from jax.sharding import Mesh, PartitionSpec, NamedSharding
    from jax.experimental.shard_map import shard_map
    from concourse import bass2jax

    bass2jax.install_neuronx_cc_hook()
    partition_name = (
        nc.partition_id_tensor.name if nc.partition_id_tensor else None
    )
    in_names, out_names, out_avals, zero_outs = [], [], [], []
    for alloc in nc.m.functions[0].allocations:
        if not isinstance(alloc, mybir.MemoryLocationSet):
            continue
        name = alloc.memorylocations[0].name
        if alloc.kind == "ExternalInput":
            if name != partition_name:
                in_names.append(name)
        elif alloc.kind == "ExternalOutput":
            shape = tuple(alloc.tensor_shape)
            dtype = mybir.dt.np(alloc.dtype)
            out_names.append(name)
            out_avals.append(jax.core.ShapedArray(shape, dtype))
            zero_outs.append(np.zeros(shape, dtype))
    n_params = len(in_names)
    n_outs = len(out_avals)
    all_in_names = list(in_names) + list(out_names)
    if partition_name is not None:
        all_in_names.append(partition_name)
    donate = tuple(range(n_params, n_params + n_outs))

    def _body(*args):
        operands = list(args)
        if partition_name is not None:
            operands.append(bass2jax.partition_id_tensor())
        outs = bass2jax._bass_exec_p.bind(
            *operands,
            out_avals=tuple(out_avals),
            in_names=tuple(all_in_names),
            out_names=tuple(out_names),
            lowering_input_output_aliases=(),
            sim_require_finite=True,
            sim_require_nnan=True,
            nc=nc,
        )
        return tuple(outs)

    devices = jax.devices()[:n_cores]
    mesh = Mesh(np.asarray(devices), ("core",))
    in_specs = (PartitionSpec("core"),) * (n_params + n_outs)
    out_specs = (PartitionSpec("core"),) * len(out_names)
    sharded = jax.jit(
        shard_map(
            _body, mesh=mesh, in_specs=in_specs, out_specs=out_specs,
            check_rep=False,
        ),
        donate_argnums=donate,
        keep_unused=True,
    )
    per_core = [
        [np.asarray(m[name]) for name in in_names] for m in in_maps
    ]
    concat_in = [
        np.concatenate([per_core[c][i] for c in range(n_cores)], axis=0)
        for i in range(n_params)
    ]
    sh = NamedSharding(mesh, PartitionSpec("core"))
    in_dev = [jax.device_put(a, sh) for a in concat_in]
    jax.block_until_ready(in_dev)

    times = []
    out_arrs = None
    for _ in range(iters):
        zeros_dev = [
            jax.device_put(
                np.zeros((n_cores * z.shape[0], *z.shape[1:]), z.dtype), sh
            )
            for z in zero_outs
        ]
        jax.block_until_ready(zeros_dev)
        t0 = _time.perf_counter()
        out_arrs = sharded(*in_dev, *zeros_dev)
        jax.block_until_ready(out_arrs)
        times.append(_time.perf_counter() - t0)

    results = [
        {
            name: np.asarray(out_arrs[i]).reshape(n_cores, *out_avals[i].shape)[c]
            for i, name in enumerate(out_names)
        }
        for c in range(n_cores)
    ]
    return results, times


def _in_maps(x, h0T, c0, Us, Vs, bs):
    maps = []
    for k in range(NCORES):
        sl = slice(k * HS, (k + 1) * HS)
        maps.append(
            {
                "x": x,
                "h0T": h0T,
                "c0s": np.ascontiguousarray(c0[:, sl]),
                "Vsel": np.ascontiguousarray(
                    np.concatenate([V[:, sl] for V in Vs], axis=1)
                ),
                "Usel": np.ascontiguousarray(
                    np.concatenate([U[:, sl] for U in Us], axis=1)
                ),
                "bsel": np.ascontiguousarray(
                    np.concatenate([b[sl] for b in bs])[None, :]
                ),
            }
        )
    return maps


def _prep(x, h0, c0, U_i, V_i, b_i, U_f, V_f, b_f, U_o, V_o, b_o, U_g, V_g, b_g):
    x = np.ascontiguousarray(np.asarray(x, dtype=np.float32))
    h0T = np.ascontiguousarray(np.asarray(h0, dtype=np.float32).T)
    c0 = np.asarray(c0, dtype=np.float32)
    Us = [np.asarray(a, dtype=np.float32) for a in (U_i, U_f, U_o, U_g)]
    Vs = [np.asarray(a, dtype=np.float32) for a in (V_i, V_f, V_o, V_g)]
    bs = [np.asarray(a, dtype=np.float32) for a in (b_i, b_f, b_o, b_g)]
    return _in_maps(x, h0T, c0, Us, Vs, bs)


def kernel_timed(inputs, iters=10, t_steps=T):
    """Run the kernel with warm-execution timing. Returns (output, times_s)."""
    in_maps = _prep(**inputs)
    if t_steps != T:
        in_maps = [
            {**m, "x": np.ascontiguousarray(m["x"][:, :t_steps, :])}
            for m in in_maps
        ]
    nc = _get_nc(t_steps)
    results, times = _timed_run(nc, in_maps, NCORES, iters=iters)
    out = np.concatenate([results[k]["hseq"] for k in range(NCORES)], axis=2)
    return out, times


def _get_nc(t_steps=T):
    if t_steps not in _cache:
        _cache[t_steps] = build(t_steps)
    return _cache[t_steps]


def kernel(x, h0, c0, U_i, V_i, b_i, U_f, V_f, b_f, U_o, V_o, b_o, U_g, V_g, b_g):
    global LAST_EXEC_NS, LAST_RESULT
    in_maps = _prep(
        x, h0, c0, U_i, V_i, b_i, U_f, V_f, b_f, U_o, V_o, b_o, U_g, V_g, b_g
    )
    nc = _get_nc(T)
    res = run_bass_kernel_spmd(nc, in_maps, list(range(NCORES)), trace=TRACE)
    LAST_EXEC_NS = res.exec_time_ns
    LAST_RESULT = res
    outs = [res.results[k]["hseq"] for k in range(NCORES)]
    return np.concatenate(outs, axis=2)
